# revision 21
# baseline (speedup 1.0000x reference)
"""Sharded masked dot-product attention for 8 TRN2 NeuronCores.

Problem: B=64, Lq=Lk=1024, D=64 fp32 attention with per-batch valid_lens
masking (scores at k >= valid_len forced to -1e6 before softmax).

Strategy
--------
Batch dim sharded 8 ways (8 batches per core, one per "slot"), batches
sorted by valid-k-block count and dealt round-robin so per-slot block
counts are tight.

Work is streamed as "units" u = (slot, k-block, q-half): S^T chunks of
[128 k, 512 q].  The PE stream is software-pipelined with the QK matmul
running SKEW=4 units ahead of the AV matmul, so the exp stage's latency
(plus ~0.24us/hop semaphore latency) stays off the PE's critical path:
PE order = ..., QK(u+4), AV(u), QK(u+5), AV(u+1), ...
PSUM: 5 S^T unit buffers (1 bank each) + 3 rotating AV accumulator
halves (1 bank each) = exactly 8 banks.

exp runs per unit on one of two engines (planned per block on the host):
  ACT: exact exp(0.125 S + bias), mask bias folded in (free).
  DVE: one-instruction Schraudolph exp -- int32(S*(0.125*2^23/ln2) +
       (127*2^23-C)) bit-cast to fp32 (~3% per-weight error, which
       mostly cancels in the softmax ratio).  Only k-blocks fully valid
       on every core of the slot are DVE-eligible (no masking needed).
       The host balances ACT/DVE busy time; both stay under the PE
       roofline.

The AV matmul accumulates V_aug^T @ A^T with a ones-column in V_aug, so
row 64 of the accumulator is the softmax denominator.  Accumulators are
copied PSUM->SBUF per q-half (h-major unit order makes half 0 finish
mid-slot, overlapping its copy+DMA with compute) and DMA'd out
unnormalized; the host divides by the denominator row and transposes.

Slots are emitted in SLOT_ORDER, rotated so the largest slots (pair 0,
with ACT/DVE-interleaved plans) come last -- they drain the pipeline
tail fastest -- and the first DMA (a small fused [mask | K head | V
block0 | Q half]) belongs to a smaller pair, starting compute sooner.
"""

import numpy as np

import concourse.mybir as mybir
import concourse.tile as tile
from concourse import bacc
from concourse.bass_utils import run_bass_kernel_spmd

B, LQ, LK, D = 64, 1024, 1024, 64
NCORES = 8
SLOTS = 8                 # batches per core
KB = 128                  # k-block size (partition dim of S^T)
NKB_MAX = LK // KB        # 8
QH = 512                  # q per matmul / unit (PSUM bank = 512 fp32)
NQH = LQ // QH            # 2
MASK_VALUE = -1000000.0
SCALE = 1.0 / np.sqrt(D)  # 0.125, folded into the exp
SKEW = 4                  # units QK leads AV by
PAIR_ORDER = (1, 2, 3, 0)
SLOT_ORDER = (2, 3, 4, 5, 6, 7, 0, 1)
HDR = 128                 # f0 header: mask bias (64) + Schraudolph bias (64)

F32 = mybir.dt.float32
I16 = mybir.dt.int16
BF16 = mybir.dt.bfloat16
MM_DT = mybir.dt.float32r

# bf16 Schraudolph constants: int16(S*K16 + bias) bit-cast to bfloat16.
# The convert rounds-to-nearest and SATURATES, so the masked-row bias
# (-2.4e7) lands on INT16_MIN = 0x8000 = bf16 -0.0 exactly.
EXP_K = float(2.0**7 / np.log(2.0)) * SCALE    # multiplies raw scores
EXP_B = float(127.0 * 2.0**7 - 5.6)
EXP_MASKED = -2.4e7

# cost-model ns for balancing the two exp engines (per unit / per copy)
_ACT_UNIT, _DVE_UNIT = 600.0, 659.0


def _plan(nkb_slot):
    """Per-block exp engine ('act' exact fp32r / 'dve' bf16 Schraudolph)
    + per-slot copy-engine pair.

    Only as many blocks go to DVE as needed to keep ACT ~10% under the
    PE roofline (fewer approximated blocks = lower output error), spread
    evenly across the emission order (Bresenham) so both exp engines
    interleave everywhere, including the drain end of the schedule.
    Copies are assigned by a forward greedy balance; the terminal copy
    goes opposite the last block's engine."""
    total = sum(nkb_slot)
    pe_ns = total * 4 * 213.0
    act_budget = 0.90 * pe_ns - SLOTS * _ACT_UNIT   # reserve ~half copies
    n_dve = min(total, max(0, total - int(act_budget // (2 * _ACT_UNIT))))
    order = [(j, kb) for j in SLOT_ORDER for kb in range(nkb_slot[j])]
    blocks = {j: [None] * nkb_slot[j] for j in range(SLOTS)}
    acc = 0
    for i, (j, kb) in enumerate(order):
        if (i + 1) * n_dve // total > acc:
            blocks[j][kb] = "dve"
            acc += 1
        else:
            blocks[j][kb] = "act"
    copies = {}
    act_ns = 0.0
    dve_ns = 0.0
    for j in SLOT_ORDER:
        for kb in range(nkb_slot[j]):
            if blocks[j][kb] == "dve":
                dve_ns += 2 * _DVE_UNIT
            else:
                act_ns += 2 * _ACT_UNIT
        ch = []
        for _ in range(2):
            if dve_ns + _DVE_UNIT < act_ns + _ACT_UNIT:
                ch.append("dve")
                dve_ns += _DVE_UNIT
            else:
                ch.append("act")
                act_ns += _ACT_UNIT
        copies[j] = ch
    last = SLOT_ORDER[-1]
    copies[last][1] = "dve" if blocks[last][-1] == "act" else "act"
    return (
        tuple(tuple(blocks[j]) for j in range(SLOTS)),
        tuple(tuple(copies[j]) for j in range(SLOTS)),
    )


def _layout(nkb_slot, blocks_plan):
    """Column offsets for the packed DRAM tensors, in PAIR_ORDER.

    V_aug is packed into two tensors by exp plan: va (fp32r, ACT blocks)
    and vb (bf16, DVE blocks), each laid out in emission order."""
    kcols = [nkb_slot[2 * p] * KB for p in range(4)]
    pf = PAIR_ORDER[0]
    kt0n = min(2, nkb_slot[2 * pf])
    kt_off = {}
    o = 0
    for i, p in enumerate(PAIR_ORDER):
        skip = kt0n * KB if i == 0 else 0
        kt_off[p] = o - skip
        o += kcols[p] - skip
    kt_total = o
    qt_off = {}
    o = 0
    for i, p in enumerate(PAIR_ORDER):
        skip = QH if i == 0 else 0
        qt_off[p] = o - skip
        o += LQ - skip
    qt_total = o
    # per-(j, kb) column offset into va (act) or vb (dve)
    va_slot = {}
    ao = 0
    bo = 0
    for j in SLOT_ORDER:
        for kb in range(nkb_slot[j]):
            if blocks_plan[j][kb] == "dve":
                va_slot[(j, kb)] = bo
                bo += D + 1
            else:
                va_slot[(j, kb)] = ao
                ao += D + 1
    f0c = HDR + kt0n * KB + QH
    return kcols, kt0n, kt_off, kt_total, qt_off, qt_total, \
        va_slot, ao, bo, f0c


def _emit(ctx, tc, aps, nkb_slot, plan, rep=0):
    nc = tc.nc
    f0_d, qt_d, kt_d, va_d, vb_d, ot_d = aps
    blocks_plan, copy_plan = plan
    (kcols, kt0n, kt_off, _, qt_off, _, va_slot, va_total, vb_total, f0c) = \
        _layout(nkb_slot, blocks_plan)
    pf = PAIR_ORDER[0]

    io = ctx.enter_context(tc.tile_pool(name=f"io{rep}", bufs=1))
    apool = ctx.enter_context(tc.tile_pool(name=f"apool{rep}", bufs=2))
    psum = ctx.enter_context(tc.tile_pool(name=f"psum{rep}", bufs=2, space="PSUM"))

    # Warm-up activation: forces the Exp table load at t=0.
    warm = io.tile([1, 1], F32, tag="warm", bufs=1)
    nc.vector.memset(warm, 0.0)
    nc.scalar.activation(out=warm, in_=warm, func=mybir.ActivationFunctionType.Exp)

    # PE warm-up: fp32r dummy matmuls on a zero scratch ramp the PE
    # p-state to full speed during the initial DMA window, so the first
    # real QK matmuls run at 2.4GHz instead of 1.2GHz.
    scr = io.tile([64, QH], BF16, tag="scr", bufs=1)
    nc.vector.memset(scr, 0.0)
    for w in range(6):
        wps = psum.tile([128, QH], F32, tag="st", bufs=5, name=f"wps{w}")
        nc.tensor.matmul(wps, lhsT=scr[:, :KB], rhs=scr, start=True, stop=True)

    # --- input DMAs, ordered so early units' operands land first --------
    # f0 = [mask bias (64) | Schraudolph bias (64) | K^T first-pair
    #       blocks 0..kt0n-1 | Q^T first-pair qh0]
    f0 = io.tile([128, f0c], MM_DT, tag="f0", bufs=1)
    nc.sync.dma_start(out=f0, in_=f0_d)
    mb_all = f0[:, 0:64].bitcast(F32)
    sb_all = f0[:, 64:HDR].bitcast(F32)
    ktr_pf = None
    if kcols[pf] > kt0n * KB:
        ktr_pf = io.tile([128, kcols[pf] - kt0n * KB], MM_DT,
                         tag="ktrpf", bufs=1)
        nc.sync.dma_start(
            out=ktr_pf, in_=kt_d[:, :kcols[pf] - kt0n * KB]
        )
    # va head = first two slots' worth, so the first AVs aren't gated on
    # the full V transfer; rest follows later in the queue.
    va_head = sum(
        D + 1
        for j in SLOT_ORDER[:2]
        for kb in range(nkb_slot[j])
        if blocks_plan[j][kb] == "act"
    )
    va_h = io.tile([128, max(va_head, 1)], MM_DT, tag="vah", bufs=1)
    if va_head:
        nc.sync.dma_start(out=va_h, in_=va_d[:, :va_head])
    vb_t = io.tile([128, max(vb_total, 1)], BF16, tag="vbt", bufs=1)
    if vb_total:
        nc.sync.dma_start(out=vb_t, in_=vb_d)
    qtfh1 = io.tile([128, QH], MM_DT, tag="qtfh1", bufs=1)
    nc.sync.dma_start(out=qtfh1, in_=qt_d[:, :QH])
    va_r = io.tile([128, max(va_total - va_head, 1)], MM_DT, tag="var",
                   bufs=1)
    if va_total - va_head:
        nc.sync.dma_start(out=va_r, in_=va_d[:, va_head:])
    kt_p = {}
    qt_p = {}
    for p in PAIR_ORDER[1:]:
        kt_p[p] = io.tile([128, kcols[p]], MM_DT, tag=f"ktp{p}", bufs=1,
                          name=f"kt_p{p}")
        nc.sync.dma_start(out=kt_p[p], in_=kt_d[:, kt_off[p]:kt_off[p] + kcols[p]])
        qt_p[p] = io.tile([128, LQ], MM_DT, tag=f"qtp{p}", bufs=1,
                          name=f"qt_p{p}")
        nc.sync.dma_start(out=qt_p[p], in_=qt_d[:, qt_off[p]:qt_off[p] + LQ])

    def kt_ap(p, kb):
        if p == pf:
            if kb < kt0n:
                return f0[:, HDR + kb * KB:HDR + (kb + 1) * KB]
            return ktr_pf[:, (kb - kt0n) * KB:(kb - kt0n + 1) * KB]
        return kt_p[p][:, kb * KB:(kb + 1) * KB]

    def qt_ap(p, h):
        if p == pf:
            return f0[:, f0c - QH:] if h == 0 else qtfh1
        return qt_p[p][:, h * QH:(h + 1) * QH]

    def va_ap(j, kb):
        off = va_slot[(j, kb)]
        if blocks_plan[j][kb] == "dve":
            return vb_t[:, off:off + (D + 1)]
        if off < va_head:
            return va_h[:, off:off + (D + 1)]
        return va_r[:, off - va_head:off - va_head + (D + 1)]

    # --- unit-granular software pipeline --------------------------------
    # h-major within each slot: all k-blocks at q-half 0, then q-half 1.
    # Each half's accumulator completes mid-slot, so its copy+DMA overlap
    # the other half's compute and the end-of-program drain is short.
    units = [
        (j, kb, h)
        for j in SLOT_ORDER
        for h in range(NQH)
        for kb in range(nkb_slot[j])
    ]
    sts = {}
    avs = {}
    ats = {}
    pending = []   # (emit_at_index, (j, h)) delayed copy+DMA

    def emit_qk(u):
        j, kb, h = u
        p, half = divmod(j, 2)
        base = 64 * half
        st = psum.tile([128, QH], F32, tag="st", bufs=5,
                       name=f"st{j}_{kb}_{h}")
        nc.tensor.matmul(
            st,
            lhsT=kt_ap(p, kb)[base:base + 64, :],
            rhs=qt_ap(p, h)[base:base + 64, :],
            start=True,
            stop=True,
        )
        sts[u] = st

    def emit_exp(u):
        j, kb, h = u
        st = sts.pop(u)
        if blocks_plan[j][kb] == "dve":
            at = apool.tile([128, QH], I16, tag="ab", bufs=8,
                            name=f"ab{j}_{kb}_{h}")
            nc.vector.tensor_scalar(
                out=at,
                in0=st,
                scalar1=EXP_K,
                scalar2=sb_all[:, j * NKB_MAX + kb:j * NKB_MAX + kb + 1],
                op0=mybir.AluOpType.mult,
                op1=mybir.AluOpType.add,
            )
        else:
            at = apool.tile([128, QH], MM_DT, tag="at", bufs=8,
                            name=f"at{j}_{kb}_{h}")
            nc.scalar.activation(
                out=at,
                in_=st,
                func=mybir.ActivationFunctionType.Exp,
                bias=mb_all[:, j * NKB_MAX + kb:j * NKB_MAX + kb + 1],
                scale=SCALE,
            )
        ats[u] = at

    def emit_av(u, i):
        j, kb, h = u
        nkb = nkb_slot[j]
        if (j, h) not in avs:
            avs[(j, h)] = psum.tile([D + 1, QH], F32, tag="av", bufs=3,
                                    name=f"av{j}_{h}")
        at = ats.pop(u)
        nc.tensor.matmul(
            avs[(j, h)],
            lhsT=va_ap(j, kb),
            rhs=at.bitcast(BF16) if blocks_plan[j][kb] == "dve" else at,
            start=(kb == 0),
            stop=(kb == nkb - 1),
        )
        if kb == nkb - 1:
            # copy+DMA a few units later so the copy (which waits on this
            # AV) never head-of-line-blocks its engine's exp queue
            pending.append((i + 3, (j, h)))

    def flush_pending(i, force=False):
        while pending and (force or pending[0][0] <= i):
            _, (j, h) = pending.pop(0)
            av = avs.pop((j, h))
            ot_t = apool.tile([D + 1, QH], F32, tag="ot", bufs=3,
                              name=f"ot{j}_{h}")
            if copy_plan[j][h] == "dve":
                nc.vector.tensor_scalar_add(ot_t, av, 0.0)
            else:
                nc.scalar.copy(out=ot_t, in_=av)
            nc.sync.dma_start(out=ot_d[j, :, h * QH:(h + 1) * QH], in_=ot_t)

    for i, u in enumerate(units):
        emit_qk(u)
        emit_exp(u)
        if i >= SKEW:
            emit_av(units[i - SKEW], i)
        flush_pending(i)
    n = len(units)
    for i in range(max(0, n - SKEW), n):
        emit_av(units[i], i + SKEW)
        flush_pending(i + SKEW)
    flush_pending(0, force=True)


def build_program(nkb_slot, plan, repeat=1):
    """Build + compile the per-core Bass program (SPMD across 8 cores).
    repeat>1 re-emits the body for slope benchmarking."""
    from contextlib import ExitStack

    nc = bacc.Bacc(
        "TRN2", target_bir_lowering=False, debug=False, num_devices=NCORES
    )
    (_, _, _, kt_total, _, qt_total, _, va_total, vb_total, f0c) = \
        _layout(nkb_slot, plan[0])
    f0 = nc.dram_tensor("f0", [128, f0c], MM_DT, kind="ExternalInput").ap()
    qt = nc.dram_tensor("qt", [128, qt_total], MM_DT, kind="ExternalInput").ap()
    kt = nc.dram_tensor("kt", [128, kt_total], MM_DT, kind="ExternalInput").ap()
    va = nc.dram_tensor("va", [128, max(va_total, 1)], MM_DT,
                        kind="ExternalInput").ap()
    vb = nc.dram_tensor("vb", [128, max(vb_total, 1)], BF16,
                        kind="ExternalInput").ap()
    ot = nc.dram_tensor("ot", [SLOTS, D + 1, LQ], F32, kind="ExternalOutput").ap()

    with tile.TileContext(nc) as tc:
        for r in range(repeat):
            with ExitStack() as ctx:
                _emit(ctx, tc, (f0, qt, kt, va, vb, ot), nkb_slot, plan, rep=r)
    nc.compile()
    return nc


def shard_inputs(queries, keys, values, valid_lens):
    """Returns (nkb_slot, plan, in_maps list, assignment array)."""
    import ml_dtypes

    queries = np.asarray(queries, dtype=np.float32)
    keys = np.asarray(keys, dtype=np.float32)
    values = np.asarray(values, dtype=np.float32)
    vl = np.asarray(valid_lens).astype(np.int64).reshape(B)
    vl = np.clip(vl, 1, LK)

    nkb = np.clip((vl + KB - 1) // KB, 1, NKB_MAX).astype(np.int64)
    order = np.argsort(-nkb, kind="stable")
    assignment = np.empty((NCORES, SLOTS), dtype=np.int64)
    for j in range(SLOTS):
        for c in range(NCORES):
            assignment[c, j] = order[j * NCORES + c]
    nkb_slot = tuple(int(nkb[order[j * NCORES]]) for j in range(SLOTS))
    plan = _plan(nkb_slot)
    blocks_plan = plan[0]

    (kcols, kt0n, kt_off, kt_total, qt_off, qt_total,
     va_slot, va_total, vb_total, f0c) = _layout(nkb_slot, blocks_plan)
    pf = PAIR_ORDER[0]

    kpos = np.arange(LK)
    in_maps = []
    for c in range(NCORES):
        f0_np = np.zeros((128, f0c), dtype=np.float32)
        qt_np = np.empty((128, qt_total), dtype=np.float32)
        kt_np = np.empty((128, kt_total), dtype=np.float32)
        va_np = np.zeros((128, max(va_total, 1)), dtype=np.float32)
        vb_np = np.zeros((128, max(vb_total, 1)), dtype=ml_dtypes.bfloat16)
        for j in range(SLOTS):
            b = assignment[c, j]
            p, half = divmod(j, 2)
            rows = slice(half * 64, (half + 1) * 64)
            qT = queries[b].T
            kT = keys[b].T
            if p == pf:
                f0_np[rows, HDR:HDR + kt0n * KB] = kT[:, :kt0n * KB]
                if kcols[p] > kt0n * KB:
                    kt_np[rows, :kcols[p] - kt0n * KB] = (
                        kT[:, kt0n * KB:kcols[p]]
                    )
                f0_np[rows, f0c - QH:] = qT[:, :QH]
                qt_np[rows, :QH] = qT[:, QH:]
            else:
                kt_np[rows, kt_off[p]:kt_off[p] + kcols[p]] = kT[:, :kcols[p]]
                qt_np[rows, qt_off[p]:qt_off[p] + LQ] = qT
            n = nkb_slot[j]
            vblk = values[b][: n * KB].reshape(n, KB, D).transpose(1, 0, 2)
            for kb in range(n):
                off = va_slot[(j, kb)]
                if blocks_plan[j][kb] == "dve":
                    vb_np[:, off:off + D] = vblk[:, kb, :]
                    vb_np[:, off + D] = 1.0
                else:
                    va_np[:, off:off + D] = vblk[:, kb, :]
                    va_np[:, off + D] = 1.0
            # per-(slot, block) bias columns: exact-exp mask bias and
            # bf16 Schraudolph bias (saturates masked rows to -0.0)
            valid = (kpos < vl[b]).reshape(NKB_MAX, KB).T
            f0_np[:, j * NKB_MAX:(j + 1) * NKB_MAX] = np.where(
                valid, np.float32(0.0), np.float32(MASK_VALUE)
            )
            f0_np[:, 64 + j * NKB_MAX:64 + (j + 1) * NKB_MAX] = np.where(
                valid, np.float32(EXP_B), np.float32(EXP_MASKED)
            )
        in_maps.append(
            {
                "f0": np.ascontiguousarray(f0_np),
                "qt": np.ascontiguousarray(qt_np),
                "kt": np.ascontiguousarray(kt_np),
                "va": np.ascontiguousarray(va_np),
                "vb": np.ascontiguousarray(vb_np),
            }
        )
    return nkb_slot, plan, in_maps, assignment


def unshard_output(results, assignment):
    out = np.empty((B, LQ, D), dtype=np.float32)
    for c in range(NCORES):
        ot = results[c]["ot"]  # [SLOTS, D+1, LQ] unnormalized + denom row
        o = ot[:, :D, :] / ot[:, D:D + 1, :]
        for j in range(SLOTS):
            out[assignment[c, j]] = o[j].T
    return out


_PROGRAM_CACHE = {}


def _get_program(nkb_slot, plan):
    key = (nkb_slot, plan)
    nc = _PROGRAM_CACHE.get(key)
    if nc is None:
        nc = build_program(nkb_slot, plan)
        _PROGRAM_CACHE[key] = nc
    return nc


def run(inputs, trace=False, **run_kwargs):
    """Shard, run on 8 cores, unshard.  Returns (output, BassKernelResults)."""
    nkb_slot, plan, in_maps, assignment = shard_inputs(**inputs)
    nc = _get_program(nkb_slot, plan)
    res = run_bass_kernel_spmd(
        nc, in_maps, core_ids=list(range(NCORES)), trace=trace, **run_kwargs
    )
    return unshard_output(res.results, assignment), res


def kernel(queries, keys, values, valid_lens):
    out, _ = run(
        {
            "queries": queries,
            "keys": keys,
            "values": values,
            "valid_lens": valid_lens,
        }
    )
    return out
